# revision 39
# baseline (speedup 1.0000x reference)
"""Causal self-attention Trainium2 kernel (Bass/Tile), 8-core SPMD.

Problem: X[2, 2048, 1024], W_qkv[1024, 3072], W_proj[1024, 1024], H=16 heads.

Sharding: core c handles batch b = c // 4 and heads h0 = 4*(c % 4) .. h0+4
(tensor-parallel over heads + data-parallel over batch). Each core computes
a partial output  out_b = Y[:, heads] @ W_proj[head rows, :]  and the host
sums the 4 partials per batch (the W_proj row-shard reduction).

Per-core device layout ("transposed attention", no P transposes needed):
  Xt  [C, T]      X[b].T, all 32 [128, 512] tiles DMAed up front
  Qt,Kt [128,2,CH] per chunk, per head-pair group g: partition = 64*(h%2)+d
  V   [128,4,260] per chunk [token-block, head*65(+ones col)] for PV lhsT
  St  = Kt_blk.T @ Qt_chunk -> [keys 128, q 512] PSUM (K=d=64 contraction),
        both heads of a pair packed side-by-side in one [128, 1024] tile
  P   = exp(0.125*(St + causal_mask))  via ACT; ones-augmented PV gives
  Yt_aug = [V|1].T @ P -> [65, q 512]: rows 0-63 = Yt, row 64 = softmax sums

Scheduling: the TRN2 PE clock ramps 0.65 -> 1.2 -> 2.4 GHz with ~3us of
CONTINUOUS execution and drops back on idle, so the whole kernel is emitted
as one dense PE stream: warmup matmuls cover the initial weight DMAs, and
the QKV projection for token chunk tch+1 plus the output projection for
chunk qi-1 are injected one group per key-block iteration into the
attention loop over chunk qi, so the PE never starves while ACT runs exp.
ST matmuls and exp skip the fully-masked columns left of the causal
diagonal; PV reads only the surviving columns.
"""

import numpy as np
from collections import deque

B, T, C, H = 2, 2048, 1024, 16
HD = 64          # head dim
HPC = 4          # heads per core
P = 128
NCORES = 8
CH = 512         # token chunk (matmul free dim / q chunk)
KB = 128         # key block
MM_DTYPE = "bf16"
WSCALE = 32.0    # host-side q/k weight scale for fp8 range


def build_nc(t_len=T, mm_dtype=None):
    import concourse.bass as bass
    import concourse.mybir as mybir
    from concourse import bacc, library_config
    from concourse.tile import TileContext
    from contextlib import ExitStack

    mm_dtype = mm_dtype or MM_DTYPE
    f32 = mybir.dt.float32
    f8 = mybir.dt.float8e4
    mdt = mybir.dt.bfloat16 if mm_dtype == "bf16" else mybir.dt.float32r
    DR = mybir.MatmulPerfMode.DoubleRow
    Exp = mybir.ActivationFunctionType.Exp
    Alu = mybir.AluOpType

    NKC = C // P          # 8 contraction chunks over C
    NCH = t_len // CH     # token chunks
    NBC = CH // P         # token blocks per chunk (4)
    NPR = NCH // 2        # token chunk PAIRS (q/k are projected per pair)
    # q/k weights are scaled by 32 on the host so their fp8 splits stay in
    # e4m3's normal range; the 32*32 cancels inside the exp scale.
    SC = 1.0 / np.sqrt(HD) / (WSCALE * WSCALE)

    nc = bacc.Bacc("TRN2", target_bir_lowering=False, debug=False,
                   num_devices=NCORES)

    xt_d = nc.dram_tensor("xt", [P, NKC, t_len], mdt, kind="ExternalInput").ap()
    xh_d = nc.dram_tensor("xh", [P, NKC, t_len], f8, kind="ExternalInput").ap()
    xl_d = nc.dram_tensor("xl", [P, NKC, t_len], f8, kind="ExternalInput").ap()
    wqkh_d = nc.dram_tensor("wqkh", [P, NKC, 4, P], f8, kind="ExternalInput").ap()
    wqkl_d = nc.dram_tensor("wqkl", [P, NKC, 4, P], f8, kind="ExternalInput").ap()
    wv_d = nc.dram_tensor("wv", [P, NKC, HPC * HD], mdt, kind="ExternalInput").ap()
    wp_d = nc.dram_tensor("wp", [P, 2, C], mdt, kind="ExternalInput").ap()
    out_d = nc.dram_tensor("out", [t_len, C], mdt, kind="ExternalOutput").ap()

    with TileContext(nc) as tc, ExitStack() as ctx:
        const = ctx.enter_context(tc.tile_pool(name="const", bufs=1))
        work = ctx.enter_context(tc.tile_pool(name="work", bufs=3))
        psm = ctx.enter_context(tc.tile_pool(name="psm", bufs=2, space="PSUM"))

        # ---- persistent SBUF tensors ----
        wqkh_sb = const.tile([P, NKC, 4, P], f8, tag="wqkh")
        wqkl_sb = const.tile([P, NKC, 4, P], f8, tag="wqkl")
        wv_sb = const.tile([P, NKC, HPC * HD], mdt, tag="wv")
        wp_sb = const.tile([P, 2, C], mdt, tag="wp")
        # per-chunk tensors (separate tiles -> no false cross-chunk deps)
        qt = [const.tile([P, 2, CH], mdt, tag=f"qt{t}", name=f"qt{t}")
              for t in range(NCH)]
        kt = [const.tile([P, 2, CH], mdt, tag=f"kt{t}", name=f"kt{t}")
              for t in range(NCH)]
        VE = HD + 8     # padded per-head V-block stride (ones, 64 dims, pad)
        va = [const.tile([P, NBC, HPC * VE], mdt, tag=f"va{t}",
                         name=f"va{t}") for t in range(NCH)]
        y2 = [const.tile([P, 2, CH], mdt, tag=f"y2{t}", name=f"y2{t}")
              for t in range(NCH)]
        xt_sb = const.tile([P, NKC, t_len], mdt, tag="xt")
        xh_sb = const.tile([P, NKC, t_len], f8, tag="xh")
        xl_sb = const.tile([P, NKC, t_len], f8, tag="xl")

        def xsrc(tch, kc, c0=0, c1=CH):
            return xt_sb[:, kc, tch * CH + c0:tch * CH + c1]
        warm = const.tile([P, CH], mdt, tag="warm")
        ones_sb = const.tile([1, HD], mdt, tag="ones")

        # ---- PE warmup: dense junk matmuls so the tensor engine p-state
        # ramps while the first weight/activation DMAs are in flight.  The
        # memset is the FIRST emitted instruction so nothing can precede it
        # in the DVE queue and stall the PE stream behind it. ----
        nc.vector.memset(warm[:], 0.0)
        nc.vector.memset(ones_sb[:], 1.0)
        wt = psm.tile([P, 2 * CH], f32, tag="st", bufs=3, name="warm")[:, :CH]
        NW = 12
        for i in range(NW):
            nc.tensor.matmul(wt[:], lhsT=warm[:, :P], rhs=warm[:],
                             start=(i == 0), stop=(i == NW - 1))
        nc.gpsimd.load_library(library_config.proxy)

        # ---- input DMAs: big tiles (2-4KB DRAM rows), ordered so the first
        # q/k DoubleRow chains and the chunk-0 v groups can start earliest.
        # Sync still issues serially (~0.6us each) but there are only 12.
        HK = NKC // 2
        nc.sync.dma_start(wqkh_sb[:, :], wqkh_d[:, :])
        nc.sync.dma_start(xh_sb[:, :HK], xh_d[:, :HK])
        nc.sync.dma_start(xt_sb[:, :HK], xt_d[:, :HK])
        nc.sync.dma_start(wv_sb[:, :], wv_d[:, :])
        nc.sync.dma_start(xh_sb[:, HK:], xh_d[:, HK:])
        nc.sync.dma_start(wqkl_sb[:, :], wqkl_d[:, :])
        nc.sync.dma_start(xl_sb[:, :HK], xl_d[:, :HK])
        nc.sync.dma_start(xl_sb[:, HK:], xl_d[:, HK:])
        nc.sync.dma_start(xt_sb[:, HK:], xt_d[:, HK:])
        nc.sync.dma_start(wp_sb[:], wp_d[:])

        # ones columns of the augmented V (softmax denominator trick).  Each
        # head's VE-stride block holds [ones, d0..d63, ones, pad]: the PV
        # lhsT window picks [0, 65) (sums land in PSUM partition 0, where
        # gpsimd partition_broadcast reads them) or [1, 66) (dims land
        # 32-aligned at partitions 0..63 with sums at 64 — the layout the
        # final group's DVE-only normalize chain needs).
        for t in range(NCH):
            for hl in range(HPC):
                nc.vector.memset(
                    va[t][:, :, hl * VE: hl * VE + 1], 1.0)
                nc.vector.memset(
                    va[t][:, :, hl * VE + HD + 1: hl * VE + HD + 2], 1.0)

        # ---- q/k projection for one token chunk: fp8 DoubleRow, 3-term
        # error-compensated split (x_hi@w_hi + x_lo@w_hi + x_hi@w_lo), two
        # kc sub-tiles per matmul.  12 DR matmuls of 256 cycles replace 8
        # bf16 matmuls of 512 cycles for the same [128, 512] q/k output
        # (matmul outputs are capped at one 2KB PSUM bank, so the chunk is
        # the widest legal N). ----
        def qk_chunk_thunks(tch, split_first=False):
            thunks = []
            pqs = {}

            def emit_terms(qk, g, terms, start, stop):
                key = (qk, g)
                if key not in pqs:
                    pqs[key] = psm.tile([P, 2 * CH], f32, tag="st", bufs=3,
                                        name=f"pq{qk}{g}")[:, :CH]
                pq = pqs[key]
                n = 0
                nt = len(terms) * HK
                for x_sb, w_sb in terms:
                    for i in range(0, NKC, 2):
                        nc.tensor.matmul(
                            pq[:],
                            lhsT=w_sb[:, i:i + 2, 2 * qk + g, :],
                            rhs=x_sb[:, i:i + 2,
                                     tch * CH:(tch + 1) * CH],
                            start=(start and n == 0),
                            stop=(stop and n == nt - 1),
                            perf_mode=DR)
                        n += 1
                if stop:
                    dst = qt if qk == 0 else kt
                    nc.vector.tensor_copy(out=dst[tch][:, g, :], in_=pq[:])

            HI = [(xh_sb, wqkh_sb)]
            LO = [(xl_sb, wqkh_sb), (xh_sb, wqkl_sb)]

            def one(qk, g, terms, start, stop):
                return lambda: emit_terms(qk, g, terms, start, stop)

            if split_first:
                # g0 q/k split into hi (ready with the first fp8 DMAs) and
                # lo parts so junk can bridge the xl/wqkl landing gap
                thunks.append(one(0, 0, HI, True, False))
                thunks.append(one(1, 0, HI, True, False))
                thunks.append(one(0, 0, LO, False, True))
                thunks.append(one(1, 0, LO, False, True))
                for g in (1,):
                    thunks.append(one(0, g, HI + LO, True, True))
                    thunks.append(one(1, g, HI + LO, True, True))
            else:
                for g in range(2):
                    thunks.append(one(0, g, HI + LO, True, True))
                    thunks.append(one(1, g, HI + LO, True, True))
            return thunks

        # ---- v projection for one token chunk: bf16, X stationary ----
        def v_group_thunks(tch):
            thunks = []

            def v_group(vb):
                def run():
                    pv = psm.tile([P, 2 * CH], f32, tag="st", bufs=3,
                                  name="pv")[:, :HPC * HD]
                    for kc in range(NKC):
                        nc.tensor.matmul(
                            pv,
                            lhsT=xsrc(tch, kc, vb * P, (vb + 1) * P),
                            rhs=wv_sb[:, kc, :],
                            start=(kc == 0), stop=(kc == NKC - 1))
                    nc.vector.tensor_copy(
                        out=va[tch][:, vb, :]
                        .rearrange("p (h e) -> p h e", e=VE)[:, :, 1:HD + 1],
                        in_=pv.rearrange("p (h e) -> p h e", e=HD))
                return run

            for vb in range(NBC):
                thunks.append(v_group(vb))
            return thunks

        # ---- output projection for one chunk: 8 injectable PE pairs.  The
        # two oc halves of a token block drain into one [128, 1024] tile and
        # leave as a single 2KB-per-row DMA (half the descriptor count and
        # full DMA-row efficiency vs two [128, 512] stores). ----
        def proj_pair_thunks(qi, ybf2=None):
            thunks = []
            last = qi == NCH - 1
            osts = [work.tile([P, 2, CH], mdt, tag="ost", bufs=4,
                              name=f"ost{qi}_{t}") for t in range(NBC)]

            def pair(tbl, oc):
                def run():
                    pp = psm.tile([P, 2 * CH], f32, tag="st", bufs=3,
                                  name="pp")[:, :CH]
                    for yc in range(2):
                        if yc == 1 and ybf2 is not None:
                            lhsT = ybf2[:, tbl * P:(tbl + 1) * P]
                        else:
                            lhsT = y2[qi][:, yc, tbl * P:(tbl + 1) * P]
                        nc.tensor.matmul(
                            pp[:],
                            lhsT=lhsT,
                            rhs=wp_sb[:, yc, oc * CH:(oc + 1) * CH],
                            start=(yc == 0), stop=(yc == 1))
                    # on the final chunk ACT is idle: alternate the PSUM
                    # drains between ACT and DVE so the pp slots free at
                    # twice the rate and the PE never waits on a drain
                    if last and oc == 0:
                        nc.scalar.copy(out=osts[tbl][:, oc, :], in_=pp[:])
                    else:
                        nc.vector.tensor_copy(out=osts[tbl][:, oc, :],
                                              in_=pp[:])
                    if oc == 1:
                        tb = qi * NBC + tbl
                        nc.sync.dma_start(out_d[tb * P:(tb + 1) * P, :],
                                          osts[tbl][:])
                return run

            for tbl in range(NBC):
                for oc in range(2):
                    thunks.append(pair(tbl, oc))
            return thunks

        # ---- normalize Yt rows by the sums rows (ytp partition 0), store
        # into y2 via partition-shifting SBUF->SBUF DMAs.  Both heads of the
        # pair share one broadcast/reciprocal/multiply over [65, 2*CH]. ----
        def normalize_pair(g, qi, ytps, last=False):
            """The PSUM tiles are drained immediately (copies) so the pool
            slots free before the reciprocal chain completes.

            last=True (latency-critical final group): the PV window put the
            dims at partitions 0..63 and the sums at 64, so the whole chain
            runs with 32-aligned partition bases: sums row bounced to
            partition 0 (DVE copy), reciprocal, rank-1 f32r PE matmul as the
            partition broadcast, and a per-head multiply straight into the
            [128, CH] proj lhsT layout — no gpsimd broadcast and no shift
            DMAs.  Returns the ybf2 tile the final proj reads as lhsT."""
            if last:
                snc = work.tile([1, 2, CH], f32, tag="snc", bufs=1)
                for hh in range(2):
                    nc.vector.tensor_copy(out=snc[:, hh, :],
                                          in_=ytps[hh][HD:HD + 1, :])
                rcp32 = work.tile([1, 2, CH], f32, tag="rcp32", bufs=1)
                nc.vector.reciprocal_approx_fast(out=rcp32[:], in_=snc[:])
                rcp = work.tile([1, 2, CH], mdt, tag="rcp", bufs=1)
                nc.vector.tensor_copy(out=rcp[:], in_=rcp32[:])
                bnc2 = work.tile([P, CH], f32, tag="bnc2", bufs=1)
                for hh in range(2):
                    nc.scalar.copy(out=bnc2[hh * HD:(hh + 1) * HD, :],
                                   in_=ytps[hh][:HD, :])
                junk(10)
                rbst = psm.tile([P, 2 * CH], f32, tag="st", bufs=3,
                                name="rbst")
                for hh in range(2):
                    nc.tensor.matmul(
                        rbst[hh * HD:(hh + 1) * HD, :CH],
                        lhsT=ones_sb[:],
                        rhs=rcp[:, hh, :],
                        start=True, stop=True)
                ybf2 = work.tile([P, CH], mdt, tag="ybf2", bufs=1)
                for hh in range(2):
                    nc.vector.tensor_tensor(
                        out=ybf2[hh * HD:(hh + 1) * HD, :],
                        in0=bnc2[hh * HD:(hh + 1) * HD, :],
                        in1=rbst[hh * HD:(hh + 1) * HD, :CH], op=Alu.mult)
                return ybf2
            bnc = work.tile([HD + 1, 2, CH], f32, tag="bounce", bufs=2)
            nc.vector.tensor_copy(out=bnc[:, 0, :], in_=ytps[0][:])
            nc.vector.tensor_copy(out=bnc[:, 1, :], in_=ytps[1][:])
            # partition_broadcast / custom-DVE ops ignore the AP base
            # partition on HW; the sums rows are at partition 0 by layout.
            rb = work.tile([HD + 1, 2, CH], f32, tag="rb", bufs=2)
            nc.gpsimd.partition_broadcast(rb[:], bnc[0:1, :, :])
            nc.vector.reciprocal_approx_fast(out=rb[:], in_=rb[:])
            ybs = work.tile([HD + 1, 2, CH], mdt, tag="ybs", bufs=2)
            # row 0 computes sums * 1/sums; only rows 1..64 are stored.
            # (DVE partition base must be aligned, so operate on [0:65].)
            nc.vector.tensor_tensor(out=ybs[:], in0=bnc[:], in1=rb[:],
                                    op=Alu.mult)
            for hh in range(2):
                # shift each head's 64 rows to partitions hh*64 .. hh*64+63
                nc.sync.dma_start(y2[qi][hh * HD:(hh + 1) * HD, g, :],
                                  ybs[1:, hh, :])
            return None

        # ---- attention for one query chunk, with PE work injection ----
        def attention(qi, queue, start_at=0):
            nkb = NBC * (qi + 1)
            it = 0
            ybf2 = None
            for g in range(2):
                fin = qi == NCH - 1 and g == 1
                # final group: window [1, 66) puts dims at partitions 0..63
                # and the sums row at 64 (see normalize_pair fast path)
                voff = 1 if fin else 0
                ytps = [psm.tile([HD + 1, CH], f32, tag="yt", bufs=2,
                                 name=f"ytp{hh}") for hh in range(2)]

                def emit_pv(pts, kb, qoff):
                    for hh in range(2):
                        hl = 2 * g + hh
                        nc.tensor.matmul(
                            ytps[hh][:, qoff:],
                            lhsT=va[kb // NBC][:, kb % NBC,
                                               hl * VE + voff:
                                               hl * VE + voff + HD + 1],
                            rhs=pts[:, hh * CH + qoff:(hh + 1) * CH],
                            start=(kb == 0), stop=(kb == nkb - 1))

                pend = deque()
                for kb in range(nkb):
                    o4 = kb - NBC * qi          # 0..3 on the diagonal band
                    qoff = max(0, o4) * KB
                    stp = psm.tile([P, 2 * CH], f32, tag="st", bufs=3,
                                   name="stp")
                    for hh in range(2):
                        nc.tensor.matmul(
                            stp[:, hh * CH + qoff:(hh + 1) * CH],
                            lhsT=kt[kb // NBC][hh * HD:(hh + 1) * HD, g,
                                               (kb % NBC) * KB:(kb % NBC + 1) * KB],
                            rhs=qt[qi][hh * HD:(hh + 1) * HD, g, qoff:],
                            start=True, stop=True)
                    # PV runs TWO blocks behind ST: half the PVs otherwise
                    # stall on the exp->select->sem chain (~6us total); the
                    # extra iteration of slack lets every PV start hot.
                    if len(pend) >= 2:
                        emit_pv(*pend.popleft())
                    if queue and it >= start_at:
                        n = -(-len(queue) // max(1, 2 * nkb - it))
                        for _ in range(min(n, len(queue))):
                            queue.popleft()()
                    it += 1
                    pts = work.tile([P, 2 * CH], mdt, tag="p", bufs=4,
                                    name="pts")
                    if o4 >= 0:
                        nc.scalar.activation(
                            out=pts.rearrange("p (h c) -> p h c", h=2)[:, :, qoff:],
                            in_=stp.rearrange("p (h c) -> p h c", h=2)[:, :, qoff:],
                            func=Exp, scale=SC)
                        # causal mask: zero the upper triangle of the
                        # 128-wide diagonal band of P (keep where q >= k,
                        # i.e. band column f >= partition p).  Runs on
                        # GpSimd so the DVE queue never delays exp or PV.
                        for hh in range(2):
                            nc.gpsimd.affine_select(
                                out=pts[:, hh * CH + qoff:hh * CH + qoff + KB],
                                in_=pts[:, hh * CH + qoff:hh * CH + qoff + KB],
                                compare_op=Alu.is_ge, fill=0.0,
                                base=0, channel_multiplier=-1,
                                pattern=[[1, KB]])
                    else:
                        nc.scalar.activation(out=pts[:], in_=stp[:],
                                             func=Exp, scale=SC)
                    pend.append((pts, kb, qoff))
                while pend:
                    emit_pv(*pend.popleft())
                r = normalize_pair(g, qi, ytps, last=fin)
                if r is not None:
                    ybf2 = r
            return ybf2

        # ---- emission schedule ----
        def junk(n):
            """Dependency-free matmuls bridging a known PE stall so the
            p-state stays at max while DMAs land."""
            jt = psm.tile([P, 2 * CH], f32, tag="st", bufs=3,
                          name="junk")[:, :CH]
            for i in range(n):
                nc.tensor.matmul(jt[:], lhsT=warm[:, :P], rhs=warm[:],
                                 start=(i == 0), stop=(i == n - 1))

        # phase A: hi halves of the chunk-0 q/k g0 chains start as soon as
        # the first fp8 DMAs land; junk bridges the xl/wqkl landing gap;
        # everything else is injected into the attention loops just ahead
        # of need.
        qk0 = qk_chunk_thunks(0, split_first=True)
        qk0[0]()
        qk0[1]()
        junk(10)
        qk0[2]()
        qk0[3]()
        carry = []
        ybf2 = None
        for qi in range(NCH):
            # carried-over proj pairs (long-ready) fill the first iterations
            # right at the chunk boundary, where the score-tile rotation
            # otherwise stalls on the exp backlog
            queue = deque(carry)
            carry = []
            if qi == 0:
                v0 = v_group_thunks(0)
                queue.extend(v0[:2])
                queue.extend(qk0[4:])
                queue.extend(v0[2:])
                queue.extend(qk_chunk_thunks(1))
                queue.extend(v_group_thunks(1))
            elif qi + 1 < NCH:
                queue.extend(qk_chunk_thunks(qi + 1))
                queue.extend(v_group_thunks(qi + 1))
            if qi > 0:
                pp = proj_pair_thunks(qi - 1)
                if qi < NCH - 1:
                    queue.extend(pp[:4])
                    carry = pp[4:]
                else:
                    queue.extend(pp)
            r = attention(qi, queue, start_at=0)
            if r is not None:
                ybf2 = r
            while queue:
                queue.popleft()()
        # final proj: yc=0 reads old y2 (ready), yc=1 reads ybf2 straight
        # from the fast normalize — the first pairs briefly wait on the
        # mult, the rest stream.  (The p-state bridge is emitted inside
        # normalize_pair's fast path.)
        junk(3)
        for th in proj_pair_thunks(NCH - 1, ybf2=ybf2):
            th()
    nc.compile()
    return nc


def _to_mm_dtype(a):
    if MM_DTYPE == "bf16":
        import ml_dtypes
        return np.ascontiguousarray(a).astype(ml_dtypes.bfloat16)
    return np.ascontiguousarray(a).astype(np.float32)


def _f8(a):
    import ml_dtypes
    return np.ascontiguousarray(a).astype(ml_dtypes.float8_e4m3fn)


def make_in_maps(X, W_qkv, W_proj, t_len=T):
    """Host-side sharding: slice + pre-arrange weights per head group,
    transpose X.  Layouts match the SBUF tensors so every DMA moves
    contiguous 1-4KB rows:
      xt/xh/xl [128, 8, T]:  [p, kc, t] = X[b].T[kc*128+p, t]
                             (xh/xl = fp8 e4m3 split of X)
      wqk hi/lo [128, 8, 4, 128]: [p, kc, 2*qk+g, m]
                             = (32*W_{q|k})[kc*128+p, g*128+m] fp8 split
      wv       [128, 8, 256]: [p, kc, m] = Wv[kc*128+p, m]
      wp       [128, 2, C]:   [64*hh+d, yc, m] = W_proj[(2*yc+hh)*64+d, m]
    """
    in_maps = []
    NKC = C // P

    def xarr(a2d):          # [C, T] -> [P, NKC, T]
        return np.ascontiguousarray(
            np.asarray(a2d).reshape(NKC, P, t_len).transpose(1, 0, 2))

    xts, xhs, xls = [], [], []
    for b in range(B):
        xt = np.asarray(X[b, :t_len, :]).T            # [C, T] f32
        xh = xt.astype(np.float32)
        xh8 = _f8(xh)
        xl8 = _f8(xh - xh8.astype(np.float32))
        xts.append(_to_mm_dtype(xarr(xt)))
        xhs.append(_f8(xarr(xh8.astype(np.float32))))
        xls.append(_f8(xarr(xl8.astype(np.float32))))

    for c in range(NCORES):
        b = c // (NCORES // B)
        h0 = HPC * (c % (NCORES // B))
        cols = slice(h0 * HD, (h0 + HPC) * HD)

        def warr(w):
            return _to_mm_dtype(
                np.ascontiguousarray(w).reshape(NKC, P, HPC * HD)
                .transpose(1, 0, 2))

        wq32 = np.asarray(W_qkv[:, cols]) * WSCALE        # [C, 256]
        wk32 = np.asarray(W_qkv[:, C:][:, cols]) * WSCALE
        wqkh = np.empty((P, NKC, 4, P), np.float32)
        wqkl = np.empty((P, NKC, 4, P), np.float32)
        for j, w in enumerate([wq32, wk32]):
            wh8 = _f8(w).astype(np.float32)
            wl8 = _f8(w - wh8).astype(np.float32)
            # [C, 256] -> [P, NKC, 2, 128]
            wqkh[:, :, 2 * j:2 * j + 2] = (
                wh8.reshape(NKC, P, 2, P).transpose(1, 0, 2, 3))
            wqkl[:, :, 2 * j:2 * j + 2] = (
                wl8.reshape(NKC, P, 2, P).transpose(1, 0, 2, 3))

        wp_c = np.ascontiguousarray(W_proj[cols, :])          # [256, C]
        wp2 = wp_c.reshape(2, 2, HD, C).transpose(1, 2, 0, 3).reshape(P, 2, C)
        in_maps.append({
            "xt": xts[b],
            "xh": xhs[b],
            "xl": xls[b],
            "wqkh": _f8(wqkh),
            "wqkl": _f8(wqkl),
            "wv": warr(W_qkv[:, 2 * C:][:, cols]),
            "wp": _to_mm_dtype(wp2),
        })
    return in_maps


_CACHE = {}
TRACE = False           # set True (e.g. from test.py) to capture an NTFF profile


def kernel(X, W_qkv, W_proj):
    import sys
    if "/opt/trn_rl_repo" not in sys.path:
        sys.path.insert(0, "/opt/trn_rl_repo")
    from concourse.bass_utils import run_bass_kernel_spmd

    X = np.asarray(X, dtype=np.float32)
    W_qkv = np.asarray(W_qkv, dtype=np.float32)
    W_proj = np.asarray(W_proj, dtype=np.float32)

    if "nc" not in _CACHE:
        _CACHE["nc"] = build_nc()
    nc = _CACHE["nc"]

    in_maps = make_in_maps(X, W_qkv, W_proj)
    res = run_bass_kernel_spmd(nc, in_maps, core_ids=list(range(NCORES)),
                               trace=TRACE)
    _CACHE["last"] = res
    out = np.empty((B, T, C), dtype=np.float32)
    ncb = NCORES // B
    for b in range(B):
        acc = res.results[b * ncb]["out"].astype(np.float32)
        for c in range(b * ncb + 1, (b + 1) * ncb):
            acc = acc + res.results[c]["out"].astype(np.float32)
        out[b] = acc
    return out



# revision 47
# speedup vs baseline: 1.1192x; 1.1192x over previous
"""Causal self-attention Trainium2 kernel (Bass/Tile), 8-core SPMD.

Problem: X[2, 2048, 1024], W_qkv[1024, 3072], W_proj[1024, 1024], H=16 heads.

Sharding: core c handles batch b = c // 4 and heads h0 = 4*(c % 4) .. h0+4
(tensor-parallel over heads + data-parallel over batch). Each core computes
a partial output  out_b = Y[:, heads] @ W_proj[head rows, :]  and the host
sums the 4 partials per batch (the W_proj row-shard reduction).

Per-core device layout ("transposed attention", no P transposes needed):
  Xt  [C, T]      X[b].T, all 32 [128, 512] tiles DMAed up front
  Qt,Kt [128,2,CH] per chunk, per head-pair group g: partition = 64*(h%2)+d
  V   [128,4,260] per chunk [token-block, head*65(+ones col)] for PV lhsT
  St  = Kt_blk.T @ Qt_chunk -> [keys 128, q 512] PSUM (K=d=64 contraction),
        both heads of a pair packed side-by-side in one [128, 1024] tile
  P   = exp(0.125*(St + causal_mask))  via ACT; ones-augmented PV gives
  Yt_aug = [V|1].T @ P -> [65, q 512]: rows 0-63 = Yt, row 64 = softmax sums

Scheduling: the TRN2 PE clock ramps 0.65 -> 1.2 -> 2.4 GHz with ~3us of
CONTINUOUS execution and drops back on idle, so the whole kernel is emitted
as one dense PE stream: warmup matmuls cover the initial weight DMAs, and
the QKV projection for token chunk tch+1 plus the output projection for
chunk qi-1 are injected one group per key-block iteration into the
attention loop over chunk qi, so the PE never starves while ACT runs exp.
ST matmuls and exp skip the fully-masked columns left of the causal
diagonal; PV reads only the surviving columns.
"""

import numpy as np
from collections import deque

B, T, C, H = 2, 2048, 1024, 16
HD = 64          # head dim
HPC = 4          # heads per core
P = 128
NCORES = 8
CH = 512         # token chunk (matmul free dim / q chunk)
KB = 128         # key block
MASK_VAL = -1.0e5
MM_DTYPE = "bf16"


def build_nc(t_len=T, mm_dtype=None):
    import concourse.bass as bass
    import concourse.mybir as mybir
    from concourse import bacc, library_config
    from concourse.tile import TileContext
    from contextlib import ExitStack

    mm_dtype = mm_dtype or MM_DTYPE
    f32 = mybir.dt.float32
    mdt = mybir.dt.bfloat16 if mm_dtype == "bf16" else mybir.dt.float32r
    Exp = mybir.ActivationFunctionType.Exp
    Alu = mybir.AluOpType

    NKC = C // P          # 8 contraction chunks over C
    NCH = t_len // CH     # token chunks
    NBC = CH // P         # token blocks per chunk (4)
    SC = 1.0 / np.sqrt(HD)

    nc = bacc.Bacc("TRN2", target_bir_lowering=False, debug=False,
                   num_devices=NCORES)

    xt_d = nc.dram_tensor("xt", [P, NKC, t_len], mdt,
                          kind="ExternalInput").ap()
    wq_d = nc.dram_tensor("wq", [P, NKC, HPC * HD], mdt, kind="ExternalInput").ap()
    wk_d = nc.dram_tensor("wk", [P, NKC, HPC * HD], mdt, kind="ExternalInput").ap()
    wv_d = nc.dram_tensor("wv", [P, NKC, HPC * HD], mdt, kind="ExternalInput").ap()
    wp_d = nc.dram_tensor("wp", [P, 2, C], mdt, kind="ExternalInput").ap()
    out_d = nc.dram_tensor("out", [t_len, C], mdt, kind="ExternalOutput").ap()

    with TileContext(nc) as tc, ExitStack() as ctx:
        const = ctx.enter_context(tc.tile_pool(name="const", bufs=1))
        work = ctx.enter_context(tc.tile_pool(name="work", bufs=3))
        psm = ctx.enter_context(tc.tile_pool(name="psm", bufs=2, space="PSUM"))

        # ---- persistent SBUF tensors ----
        wq_sb = const.tile([P, NKC, HPC * HD], mdt, tag="wq")
        wk_sb = const.tile([P, NKC, HPC * HD], mdt, tag="wk")
        wv_sb = const.tile([P, NKC, HPC * HD], mdt, tag="wv")
        wp_sb = const.tile([P, 2, C], mdt, tag="wp")
        # per-chunk tensors (separate tiles -> no false cross-chunk deps)
        qt = [const.tile([P, 2, CH], mdt, tag=f"qt{t}", name=f"qt{t}")
              for t in range(NCH)]
        kt = [const.tile([P, 2, CH], mdt, tag=f"kt{t}", name=f"kt{t}")
              for t in range(NCH)]
        VE = HD + 8     # padded per-head V-block stride (ones, 64 dims, pad)
        va = [const.tile([P, NBC, HPC * VE], mdt, tag=f"va{t}",
                         name=f"va{t}") for t in range(NCH)]
        y2 = [const.tile([P, 2, CH], mdt, tag=f"y2{t}", name=f"y2{t}")
              for t in range(NCH)]
        # one big Xt tile, DMAed in 4 token-quarter slabs of [128, 8, 512]
        # (one issue each instead of 8; 1KB DRAM rows)
        xt_sb = const.tile([P, NKC, t_len], mdt, tag="xt")

        def xsrc(tch, kc, c0=0, c1=CH):
            return xt_sb[:, kc, tch * CH + c0:tch * CH + c1]
        warm = const.tile([P, CH], mdt, tag="warm")
        ones_sb = const.tile([1, HD], mdt, tag="ones")

        # ---- PE warmup: dense junk matmuls so the tensor engine p-state
        # ramps while the first weight/activation DMAs are in flight.  The
        # memset is the FIRST emitted instruction so nothing can precede it
        # in the DVE queue and stall the PE stream behind it. ----
        nc.vector.memset(warm[:], 0.0)
        nc.vector.memset(ones_sb[:], 1.0)
        wt = psm.tile([P, 2 * CH], f32, tag="st", bufs=3, name="warm")[:, :CH]
        NW = 10
        for i in range(NW):
            nc.tensor.matmul(wt[:], lhsT=warm[:, :P], rhs=warm[:],
                             start=(i == 0), stop=(i == NW - 1))
        nc.gpsimd.load_library(library_config.proxy)

        # ---- input DMAs: 8 big-tile issues in consumption order.  Each
        # token-quarter slab of Xt lands as one transfer, so qkv(tch) sees
        # its whole contraction at once; Sync's serial ~0.6us issue cost
        # drops from 87 DMAs to 8. ----
        nc.sync.dma_start(wq_sb[:], wq_d[:])
        nc.sync.dma_start(xt_sb[:, :, 0:CH], xt_d[:, :, 0:CH])
        nc.sync.dma_start(wk_sb[:], wk_d[:])
        nc.sync.dma_start(wv_sb[:], wv_d[:])
        for tch in range(1, NCH):
            nc.sync.dma_start(xt_sb[:, :, tch * CH:(tch + 1) * CH],
                              xt_d[:, :, tch * CH:(tch + 1) * CH])
        nc.sync.dma_start(wp_sb[:], wp_d[:])

        # ones columns of the augmented V (softmax denominator trick).  Each
        # head's VE-stride block holds [ones, d0..d63, ones, pad]: the PV
        # lhsT window picks [0, 65) (sums land in PSUM partition 0, where
        # gpsimd partition_broadcast reads them) or [1, 66) (dims land
        # 32-aligned at partitions 0..63 with sums at 64 — the layout the
        # final group's DVE-only normalize chain needs).
        for t in range(NCH):
            for hl in range(HPC):
                nc.vector.memset(
                    va[t][:, :, hl * VE: hl * VE + 1], 1.0)
                nc.vector.memset(
                    va[t][:, :, hl * VE + HD + 1: hl * VE + HD + 2], 1.0)

        # ---- QKV projection for one token chunk: 8 injectable PE groups ----
        def qkv_group_thunks(tch):
            thunks = []

            def qk_group(w_sb, dst, g):
                def run():
                    pq = psm.tile([P, 2 * CH], f32, tag="st", bufs=3,
                                  name="pq")[:, :CH]
                    for kc in range(NKC):
                        nc.tensor.matmul(
                            pq[:],
                            lhsT=w_sb[:, kc, g * P:(g + 1) * P],
                            rhs=xsrc(tch, kc),
                            start=(kc == 0), stop=(kc == NKC - 1))
                    nc.vector.tensor_copy(out=dst[tch][:, g, :], in_=pq[:])
                return run

            def v_group(vb):
                def run():
                    pv = psm.tile([P, 2 * CH], f32, tag="st", bufs=3,
                                  name="pv")[:, :HPC * HD]
                    for kc in range(NKC):
                        nc.tensor.matmul(
                            pv,
                            lhsT=xsrc(tch, kc, vb * P, (vb + 1) * P),
                            rhs=wv_sb[:, kc, :],
                            start=(kc == 0), stop=(kc == NKC - 1))
                    nc.vector.tensor_copy(
                        out=va[tch][:, vb, :]
                        .rearrange("p (h e) -> p h e", e=VE)[:, :, 1:HD + 1],
                        in_=pv.rearrange("p (h e) -> p h e", e=HD))
                return run

            for g in range(2):
                thunks.append(qk_group(wq_sb, qt, g))
                thunks.append(qk_group(wk_sb, kt, g))
            for vb in range(NBC):
                thunks.append(v_group(vb))
            return thunks

        # ---- output projection for one chunk: 8 injectable PE pairs.  The
        # two oc halves of a token block drain into one [128, 1024] tile and
        # leave as a single 2KB-per-row DMA (half the descriptor count and
        # full DMA-row efficiency vs two [128, 512] stores). ----
        def proj_pair_thunks(qi, ybf2=None):
            thunks = []
            last = qi == NCH - 1
            osts = [work.tile([P, 2, CH], mdt, tag="ost", bufs=4,
                              name=f"ost{qi}_{t}") for t in range(NBC)]

            def pair(tbl, oc):
                def run():
                    pp = psm.tile([P, 2 * CH], f32, tag="st", bufs=3,
                                  name="pp")[:, :CH]
                    for yc in range(2):
                        if yc == 1 and ybf2 is not None:
                            lhsT = ybf2[:, tbl * P:(tbl + 1) * P]
                        else:
                            lhsT = y2[qi][:, yc, tbl * P:(tbl + 1) * P]
                        nc.tensor.matmul(
                            pp[:],
                            lhsT=lhsT,
                            rhs=wp_sb[:, yc, oc * CH:(oc + 1) * CH],
                            start=(yc == 0), stop=(yc == 1))
                    # on the final chunk ACT is idle: alternate the PSUM
                    # drains between ACT and DVE so the pp slots free at
                    # twice the rate and the PE never waits on a drain
                    if last and oc == 0:
                        nc.scalar.copy(out=osts[tbl][:, oc, :], in_=pp[:])
                    else:
                        nc.vector.tensor_copy(out=osts[tbl][:, oc, :],
                                              in_=pp[:])
                    if oc == 1:
                        tb = qi * NBC + tbl
                        if last and tbl >= 2:
                            # the very last outputs bound the kernel tail at
                            # per-queue DMA drain rate (~40GB/s): split them
                            # across two ring queues
                            nc.sync.dma_start(
                                out_d[tb * P:(tb + 1) * P, :CH],
                                osts[tbl][:, 0, :])
                            nc.sync.dma_start(
                                out_d[tb * P:(tb + 1) * P, CH:],
                                osts[tbl][:, 1, :])
                        else:
                            nc.sync.dma_start(out_d[tb * P:(tb + 1) * P, :],
                                              osts[tbl][:])
                return run

            for tbl in range(NBC):
                for oc in range(2):
                    thunks.append(pair(tbl, oc))
            return thunks

        # ---- normalize Yt rows by the sums rows (ytp partition 0), store
        # into y2 via partition-shifting SBUF->SBUF DMAs.  Both heads of the
        # pair share one broadcast/reciprocal/multiply over [65, 2*CH]. ----
        def normalize_pair(g, qi, ytps, last=False):
            """The PSUM tiles are drained immediately (copies) so the pool
            slots free before the reciprocal chain completes.

            last=True (latency-critical final group): the PV window put the
            dims at partitions 0..63 and the sums at 64, so the whole chain
            runs with 32-aligned partition bases: sums row bounced to
            partition 0 (DVE copy), reciprocal, rank-1 f32r PE matmul as the
            partition broadcast, and a per-head multiply straight into the
            [128, CH] proj lhsT layout — no gpsimd broadcast and no shift
            DMAs.  Returns the ybf2 tile the final proj reads as lhsT."""
            if last:
                snc = work.tile([1, 2, CH], f32, tag="snc", bufs=1)
                for hh in range(2):
                    nc.vector.tensor_copy(out=snc[:, hh, :],
                                          in_=ytps[hh][HD:HD + 1, :])
                rcp32 = work.tile([1, 2, CH], f32, tag="rcp32", bufs=1)
                nc.vector.reciprocal_approx_fast(out=rcp32[:], in_=snc[:])
                rcp = work.tile([1, 2, CH], mdt, tag="rcp", bufs=1)
                nc.vector.tensor_copy(out=rcp[:], in_=rcp32[:])
                bnc2 = work.tile([P, CH], f32, tag="bnc2", bufs=1)
                for hh in range(2):
                    nc.scalar.copy(out=bnc2[hh * HD:(hh + 1) * HD, :],
                                   in_=ytps[hh][:HD, :])
                junk(10)
                rbst = psm.tile([P, 2 * CH], f32, tag="st", bufs=3,
                                name="rbst")
                for hh in range(2):
                    nc.tensor.matmul(
                        rbst[hh * HD:(hh + 1) * HD, :CH],
                        lhsT=ones_sb[:],
                        rhs=rcp[:, hh, :],
                        start=True, stop=True)
                ybf2 = work.tile([P, CH], mdt, tag="ybf2", bufs=1)
                for hh in range(2):
                    nc.vector.tensor_tensor(
                        out=ybf2[hh * HD:(hh + 1) * HD, :],
                        in0=bnc2[hh * HD:(hh + 1) * HD, :],
                        in1=rbst[hh * HD:(hh + 1) * HD, :CH], op=Alu.mult)
                return ybf2
            bnc = work.tile([HD + 1, 2, CH], f32, tag="bounce", bufs=2)
            nc.vector.tensor_copy(out=bnc[:, 0, :], in_=ytps[0][:])
            nc.vector.tensor_copy(out=bnc[:, 1, :], in_=ytps[1][:])
            # partition_broadcast / custom-DVE ops ignore the AP base
            # partition on HW; the sums rows are at partition 0 by layout.
            rb = work.tile([HD + 1, 2, CH], f32, tag="rb", bufs=2)
            nc.gpsimd.partition_broadcast(rb[:], bnc[0:1, :, :])
            nc.vector.reciprocal_approx_fast(out=rb[:], in_=rb[:])
            ybs = work.tile([HD + 1, 2, CH], mdt, tag="ybs", bufs=2)
            # row 0 computes sums * 1/sums; only rows 1..64 are stored.
            # (DVE partition base must be aligned, so operate on [0:65].)
            nc.vector.tensor_tensor(out=ybs[:], in0=bnc[:], in1=rb[:],
                                    op=Alu.mult)
            for hh in range(2):
                # shift each head's 64 rows to partitions hh*64 .. hh*64+63
                nc.sync.dma_start(y2[qi][hh * HD:(hh + 1) * HD, g, :],
                                  ybs[1:, hh, :])
            return None

        # ---- attention for one query chunk, with PE work injection ----
        def attention(qi, queue, start_at=0):
            nkb = NBC * (qi + 1)
            it = 0
            ybf2 = None
            for g in range(2):
                fin = qi == NCH - 1 and g == 1
                # final group: window [1, 66) puts dims at partitions 0..63
                # and the sums row at 64 (see normalize_pair fast path)
                voff = 1 if fin else 0
                ytps = [psm.tile([HD + 1, CH], f32, tag="yt", bufs=2,
                                 name=f"ytp{hh}") for hh in range(2)]

                def emit_pv(pts, kb, qoff):
                    for hh in range(2):
                        hl = 2 * g + hh
                        nc.tensor.matmul(
                            ytps[hh][:, qoff:],
                            lhsT=va[kb // NBC][:, kb % NBC,
                                               hl * VE + voff:
                                               hl * VE + voff + HD + 1],
                            rhs=pts[:, hh * CH + qoff:(hh + 1) * CH],
                            start=(kb == 0), stop=(kb == nkb - 1))

                pend = deque()
                for kb in range(nkb):
                    o4 = kb - NBC * qi          # 0..3 on the diagonal band
                    qoff = max(0, o4) * KB
                    stp = psm.tile([P, 2 * CH], f32, tag="st", bufs=3,
                                   name="stp")
                    for hh in range(2):
                        nc.tensor.matmul(
                            stp[:, hh * CH + qoff:(hh + 1) * CH],
                            lhsT=kt[kb // NBC][hh * HD:(hh + 1) * HD, g,
                                               (kb % NBC) * KB:(kb % NBC + 1) * KB],
                            rhs=qt[qi][hh * HD:(hh + 1) * HD, g, qoff:],
                            start=True, stop=True)
                    # PV runs TWO blocks behind ST: half the PVs otherwise
                    # stall on the exp->select->sem chain (~6us total); the
                    # extra iteration of slack lets every PV start hot.
                    if len(pend) >= 2:
                        emit_pv(*pend.popleft())
                    if queue and it >= start_at:
                        # on the final chunk, drain the queue ~8 iterations
                        # early so the DVE is clear of proj drains when the
                        # latency-critical fast-normalize chain starts
                        dby = 2 * nkb - (8 if qi == NCH - 1 else 0)
                        n = -(-len(queue) // max(1, dby - it))
                        for _ in range(min(n, len(queue))):
                            queue.popleft()()
                    it += 1
                    pts = work.tile([P, 2 * CH], mdt, tag="p", bufs=4,
                                    name="pts")
                    if o4 >= 0:
                        nc.scalar.activation(
                            out=pts.rearrange("p (h c) -> p h c", h=2)[:, :, qoff:],
                            in_=stp.rearrange("p (h c) -> p h c", h=2)[:, :, qoff:],
                            func=Exp, scale=SC)
                        # causal mask: zero the upper triangle of the
                        # 128-wide diagonal band of P (keep where q >= k,
                        # i.e. band column f >= partition p).  Runs on
                        # GpSimd so the DVE queue never delays exp or PV.
                        for hh in range(2):
                            nc.gpsimd.affine_select(
                                out=pts[:, hh * CH + qoff:hh * CH + qoff + KB],
                                in_=pts[:, hh * CH + qoff:hh * CH + qoff + KB],
                                compare_op=Alu.is_ge, fill=0.0,
                                base=0, channel_multiplier=-1,
                                pattern=[[1, KB]])
                    else:
                        nc.scalar.activation(out=pts[:], in_=stp[:],
                                             func=Exp, scale=SC)
                    pend.append((pts, kb, qoff))
                while pend:
                    emit_pv(*pend.popleft())
                r = normalize_pair(g, qi, ytps, last=fin)
                if r is not None:
                    ybf2 = r
            return ybf2

        # ---- emission schedule ----
        def junk(n):
            """Dependency-free matmuls bridging a known PE stall so the
            p-state stays at max while DMAs land."""
            jt = psm.tile([P, 2 * CH], f32, tag="st", bufs=3,
                          name="junk")[:, :CH]
            for i in range(n):
                nc.tensor.matmul(jt[:], lhsT=warm[:, :P], rhs=warm[:],
                                 start=(i == 0), stop=(i == n - 1))

        # phase A: only q-g0/k-g0 of chunk 0 up front; the v-groups and the
        # g1 projections are injected into attention(0) just ahead of need.
        # The initial weight/Xt DMAs land slower than the PE consumes, so
        # junk bridges keep the clock hot between the first groups.
        t0 = qkv_group_thunks(0)
        t0[0]()
        junk(12)
        t0[1]()
        junk(12)
        carry = []
        for qi in range(NCH):
            # carried-over proj pairs (long-ready) fill the first iterations
            # right at the chunk boundary, where the score-tile rotation
            # otherwise stalls on the exp backlog
            queue = deque(carry)
            carry = []
            if qi == 0:
                queue.extend([t0[4], t0[5], t0[2], t0[3], t0[6], t0[7]])
            if qi + 1 < NCH:
                queue.extend(qkv_group_thunks(qi + 1))
            if qi > 0:
                pp = proj_pair_thunks(qi - 1)
                if qi < NCH - 1:
                    queue.extend(pp[:4])
                    carry = pp[4:]
                else:
                    queue.extend(pp)
            ybf2 = attention(qi, queue, start_at=0)
            while queue:
                queue.popleft()()
        # final proj: yc=0 reads old y2 (ready), yc=1 reads ybf2 straight
        # from the fast normalize — the first pairs briefly wait on the
        # mult, the rest stream.  (The p-state bridge is emitted inside
        # normalize_pair's fast path.)
        junk(3)
        for th in proj_pair_thunks(NCH - 1, ybf2=ybf2):
            th()
    nc.compile()
    return nc


def _to_mm_dtype(a):
    if MM_DTYPE == "bf16":
        import ml_dtypes
        return np.ascontiguousarray(a).astype(ml_dtypes.bfloat16)
    return np.ascontiguousarray(a).astype(np.float32)


def make_in_maps(X, W_qkv, W_proj, t_len=T):
    """Host-side sharding: slice + pre-arrange weights per head group,
    transpose X.  Layouts match the SBUF tensors so every weight DMA is
    fully contiguous:
      wq/wk/wv [128, 8, 256]: [p, kc, m] = W[kc*128+p, cols][m]
      wp       [128, 2, C]:   [64*hh+d, yc, m] = W_proj[(2*yc+hh)*64+d, m]
    """
    in_maps = []
    NKC = C // P
    xts = [_to_mm_dtype(np.asarray(X[b, :t_len, :]).T
                        .reshape(NKC, P, t_len).transpose(1, 0, 2))
           for b in range(B)]
    for c in range(NCORES):
        b = c // (NCORES // B)
        h0 = HPC * (c % (NCORES // B))
        cols = slice(h0 * HD, (h0 + HPC) * HD)

        def warr(w):
            return _to_mm_dtype(
                np.ascontiguousarray(w).reshape(NKC, P, HPC * HD)
                .transpose(1, 0, 2))

        wp_c = np.ascontiguousarray(W_proj[cols, :])          # [256, C]
        wp2 = wp_c.reshape(2, 2, HD, C).transpose(1, 2, 0, 3).reshape(P, 2, C)
        in_maps.append({
            "xt": xts[b],
            "wq": warr(W_qkv[:, cols]),
            "wk": warr(W_qkv[:, C:][:, cols]),
            "wv": warr(W_qkv[:, 2 * C:][:, cols]),
            "wp": _to_mm_dtype(wp2),
        })
    return in_maps


_CACHE = {}
TRACE = False           # set True (e.g. from test.py) to capture an NTFF profile


def kernel(X, W_qkv, W_proj):
    import sys
    if "/opt/trn_rl_repo" not in sys.path:
        sys.path.insert(0, "/opt/trn_rl_repo")
    from concourse.bass_utils import run_bass_kernel_spmd

    X = np.asarray(X, dtype=np.float32)
    W_qkv = np.asarray(W_qkv, dtype=np.float32)
    W_proj = np.asarray(W_proj, dtype=np.float32)

    if "nc" not in _CACHE:
        _CACHE["nc"] = build_nc()
    nc = _CACHE["nc"]

    in_maps = make_in_maps(X, W_qkv, W_proj)
    res = run_bass_kernel_spmd(nc, in_maps, core_ids=list(range(NCORES)),
                               trace=TRACE)
    _CACHE["last"] = res
    out = np.empty((B, T, C), dtype=np.float32)
    ncb = NCORES // B
    for b in range(B):
        acc = res.results[b * ncb]["out"].astype(np.float32)
        for c in range(b * ncb + 1, (b + 1) * ncb):
            acc = acc + res.results[c]["out"].astype(np.float32)
        out[b] = acc
    return out



# revision 50
# speedup vs baseline: 1.1391x; 1.0178x over previous
"""Causal self-attention Trainium2 kernel (Bass/Tile), 8-core SPMD.

Problem: X[2, 2048, 1024], W_qkv[1024, 3072], W_proj[1024, 1024], H=16 heads.

Sharding: core c handles batch b = c // 4 and heads h0 = 4*(c % 4) .. h0+4
(tensor-parallel over heads + data-parallel over batch). Each core computes
a partial output  out_b = Y[:, heads] @ W_proj[head rows, :]  and the host
sums the 4 partials per batch (the W_proj row-shard reduction).

Per-core device layout ("transposed attention", no P transposes needed):
  Xt  [C, T]      X[b].T, all 32 [128, 512] tiles DMAed up front
  Qt,Kt [128,2,CH] per chunk, per head-pair group g: partition = 64*(h%2)+d
  V   [128,4,260] per chunk [token-block, head*65(+ones col)] for PV lhsT
  St  = Kt_blk.T @ Qt_chunk -> [keys 128, q 512] PSUM (K=d=64 contraction),
        both heads of a pair packed side-by-side in one [128, 1024] tile
  P   = exp(0.125*(St + causal_mask))  via ACT; ones-augmented PV gives
  Yt_aug = [V|1].T @ P -> [65, q 512]: rows 0-63 = Yt, row 64 = softmax sums

Scheduling: the TRN2 PE clock ramps 0.65 -> 1.2 -> 2.4 GHz with ~3us of
CONTINUOUS execution and drops back on idle, so the whole kernel is emitted
as one dense PE stream: warmup matmuls cover the initial weight DMAs, and
the QKV projection for token chunk tch+1 plus the output projection for
chunk qi-1 are injected one group per key-block iteration into the
attention loop over chunk qi, so the PE never starves while ACT runs exp.
ST matmuls and exp skip the fully-masked columns left of the causal
diagonal; PV reads only the surviving columns.
"""

import numpy as np
from collections import deque

B, T, C, H = 2, 2048, 1024, 16
HD = 64          # head dim
HPC = 4          # heads per core
P = 128
NCORES = 8
CH = 512         # token chunk (matmul free dim / q chunk)
KB = 128         # key block
MASK_VAL = -1.0e5
MM_DTYPE = "bf16"


def build_nc(t_len=T, mm_dtype=None):
    import concourse.bass as bass
    import concourse.mybir as mybir
    from concourse import bacc, library_config
    from concourse.tile import TileContext
    from contextlib import ExitStack

    mm_dtype = mm_dtype or MM_DTYPE
    f32 = mybir.dt.float32
    mdt = mybir.dt.bfloat16 if mm_dtype == "bf16" else mybir.dt.float32r
    Exp = mybir.ActivationFunctionType.Exp
    Alu = mybir.AluOpType

    NKC = C // P          # 8 contraction chunks over C
    NCH = t_len // CH     # token chunks
    NBC = CH // P         # token blocks per chunk (4)
    SC = 1.0 / np.sqrt(HD)

    nc = bacc.Bacc("TRN2", target_bir_lowering=False, debug=False,
                   num_devices=NCORES)

    xt_d = nc.dram_tensor("xt", [P, NKC, t_len], mdt,
                          kind="ExternalInput").ap()
    wq_d = nc.dram_tensor("wq", [P, NKC, HPC * HD], mdt, kind="ExternalInput").ap()
    wk_d = nc.dram_tensor("wk", [P, NKC, HPC * HD], mdt, kind="ExternalInput").ap()
    wv_d = nc.dram_tensor("wv", [P, NKC, HPC * HD], mdt, kind="ExternalInput").ap()
    wp_d = nc.dram_tensor("wp", [P, 2, C], mdt, kind="ExternalInput").ap()
    out_d = nc.dram_tensor("out", [t_len, C], mdt, kind="ExternalOutput").ap()

    with TileContext(nc) as tc, ExitStack() as ctx:
        const = ctx.enter_context(tc.tile_pool(name="const", bufs=1))
        work = ctx.enter_context(tc.tile_pool(name="work", bufs=3))
        psm = ctx.enter_context(tc.tile_pool(name="psm", bufs=2, space="PSUM"))

        # ---- persistent SBUF tensors ----
        wq_sb = const.tile([P, NKC, HPC * HD], mdt, tag="wq")
        wk_sb = const.tile([P, NKC, HPC * HD], mdt, tag="wk")
        wv_sb = const.tile([P, NKC, HPC * HD], mdt, tag="wv")
        wp_sb = const.tile([P, 2, C], mdt, tag="wp")
        # per-chunk tensors (separate tiles -> no false cross-chunk deps)
        qt = [const.tile([P, 2, CH], mdt, tag=f"qt{t}", name=f"qt{t}")
              for t in range(NCH)]
        kt = [const.tile([P, 2, CH], mdt, tag=f"kt{t}", name=f"kt{t}")
              for t in range(NCH)]
        VE = HD + 8     # padded per-head V-block stride (ones, 64 dims, pad)
        va = [const.tile([P, NBC, HPC * VE], mdt, tag=f"va{t}",
                         name=f"va{t}") for t in range(NCH)]
        y2 = [const.tile([P, 2, CH], mdt, tag=f"y2{t}", name=f"y2{t}")
              for t in range(NCH)]
        xts = [[const.tile([P, CH], mdt, tag=f"xt{t}_{kc}",
                           name=f"xt{t}_{kc}")
                for kc in range(NKC)] for t in range(NCH)]

        def xsrc(tch, kc, c0=0, c1=CH):
            return xts[tch][kc][:, c0:c1]
        warm = const.tile([P, CH], mdt, tag="warm")
        ones_sb = const.tile([1, HD], mdt, tag="ones")

        # ---- PE warmup: dense junk matmuls so the tensor engine p-state
        # ramps while the first weight/activation DMAs are in flight.  The
        # memset is the FIRST emitted instruction so nothing can precede it
        # in the DVE queue and stall the PE stream behind it. ----
        nc.vector.memset(warm[:], 0.0)
        nc.vector.memset(ones_sb[:], 1.0)
        wt = psm.tile([P, 2 * CH], f32, tag="st", bufs=3, name="warm")[:, :CH]
        NW = 18
        for i in range(NW):
            nc.tensor.matmul(wt[:], lhsT=warm[:, :P], rhs=warm[:],
                             start=(i == 0), stop=(i == NW - 1))
        nc.gpsimd.load_library(library_config.proxy)

        # ---- input DMAs: wq + chunk-0 Xt first so QKV(0) can start; wk/wv
        # issued early (Sync issues DMAs serially at ~0.6us each and the
        # k/v matmul groups need them before the Xt tail) ----
        nc.sync.dma_start(wq_sb[:, :NKC // 2], wq_d[:, :NKC // 2])
        nc.sync.dma_start(xts[0][0][:], xt_d[:, 0, 0:CH])
        nc.sync.dma_start(wq_sb[:, NKC // 2:], wq_d[:, NKC // 2:])
        nc.sync.dma_start(xts[0][1][:], xt_d[:, 1, 0:CH])
        nc.sync.dma_start(wk_sb[:, :NKC // 2], wk_d[:, :NKC // 2])
        nc.sync.dma_start(wk_sb[:, NKC // 2:], wk_d[:, NKC // 2:])
        for kc in range(2, NKC):
            nc.sync.dma_start(xts[0][kc][:], xt_d[:, kc, 0:CH])
        nc.sync.dma_start(wv_sb[:, :NKC // 2], wv_d[:, :NKC // 2])
        nc.sync.dma_start(wv_sb[:, NKC // 2:], wv_d[:, NKC // 2:])
        nc.sync.dma_start(wp_sb[:], wp_d[:])
        for tch in range(1, NCH):
            for kc in range(NKC):
                nc.sync.dma_start(xts[tch][kc][:],
                                  xt_d[:, kc, tch * CH:(tch + 1) * CH])

        # ones columns of the augmented V (softmax denominator trick).  Each
        # head's VE-stride block holds [ones, d0..d63, ones, pad]: the PV
        # lhsT window picks [0, 65) (sums land in PSUM partition 0, where
        # gpsimd partition_broadcast reads them) or [1, 66) (dims land
        # 32-aligned at partitions 0..63 with sums at 64 — the layout the
        # final group's DVE-only normalize chain needs).
        for t in range(NCH):
            for hl in range(HPC):
                nc.vector.memset(
                    va[t][:, :, hl * VE: hl * VE + 1], 1.0)
                nc.vector.memset(
                    va[t][:, :, hl * VE + HD + 1: hl * VE + HD + 2], 1.0)

        # ---- QKV projection for one token chunk: 8 injectable PE groups ----
        def qkv_group_thunks(tch):
            thunks = []

            def qk_group(w_sb, dst, g):
                def run():
                    pq = psm.tile([P, 2 * CH], f32, tag="st", bufs=3,
                                  name="pq")[:, :CH]
                    for kc in range(NKC):
                        nc.tensor.matmul(
                            pq[:],
                            lhsT=w_sb[:, kc, g * P:(g + 1) * P],
                            rhs=xsrc(tch, kc),
                            start=(kc == 0), stop=(kc == NKC - 1))
                    nc.vector.tensor_copy(out=dst[tch][:, g, :], in_=pq[:])
                return run

            def v_group(vb):
                def run():
                    pv = psm.tile([P, 2 * CH], f32, tag="st", bufs=3,
                                  name="pv")[:, :HPC * HD]
                    for kc in range(NKC):
                        nc.tensor.matmul(
                            pv,
                            lhsT=xsrc(tch, kc, vb * P, (vb + 1) * P),
                            rhs=wv_sb[:, kc, :],
                            start=(kc == 0), stop=(kc == NKC - 1))
                    nc.vector.tensor_copy(
                        out=va[tch][:, vb, :]
                        .rearrange("p (h e) -> p h e", e=VE)[:, :, 1:HD + 1],
                        in_=pv.rearrange("p (h e) -> p h e", e=HD))
                return run

            for g in range(2):
                thunks.append(qk_group(wq_sb, qt, g))
                thunks.append(qk_group(wk_sb, kt, g))
            for vb in range(NBC):
                thunks.append(v_group(vb))
            return thunks

        # ---- output projection for one chunk: 8 injectable PE pairs.  The
        # two oc halves of a token block drain into one [128, 1024] tile and
        # leave as a single 2KB-per-row DMA (half the descriptor count and
        # full DMA-row efficiency vs two [128, 512] stores). ----
        def proj_pair_thunks(qi, ybf2=None):
            thunks = []
            last = qi == NCH - 1
            osts = [work.tile([P, 2, CH], mdt, tag="ost", bufs=4,
                              name=f"ost{qi}_{t}") for t in range(NBC)]

            def pair(tbl, oc):
                def run():
                    pp = psm.tile([P, 2 * CH], f32, tag="st", bufs=3,
                                  name="pp")[:, :CH]
                    for yc in range(2):
                        if yc == 1 and ybf2 is not None:
                            lhsT = ybf2[:, tbl * P:(tbl + 1) * P]
                        else:
                            lhsT = y2[qi][:, yc, tbl * P:(tbl + 1) * P]
                        nc.tensor.matmul(
                            pp[:],
                            lhsT=lhsT,
                            rhs=wp_sb[:, yc, oc * CH:(oc + 1) * CH],
                            start=(yc == 0), stop=(yc == 1))
                    # on the final chunk ACT is idle: alternate the PSUM
                    # drains between ACT and DVE so the pp slots free at
                    # twice the rate and the PE never waits on a drain
                    if last and oc == 0:
                        nc.scalar.copy(out=osts[tbl][:, oc, :], in_=pp[:])
                    else:
                        nc.vector.tensor_copy(out=osts[tbl][:, oc, :],
                                              in_=pp[:])
                    if oc == 1:
                        tb = qi * NBC + tbl
                        if last and tbl >= 2:
                            # the very last outputs bound the kernel tail at
                            # per-queue DMA drain rate (~40GB/s): split them
                            # across two ring queues
                            nc.sync.dma_start(
                                out_d[tb * P:(tb + 1) * P, :CH],
                                osts[tbl][:, 0, :])
                            nc.sync.dma_start(
                                out_d[tb * P:(tb + 1) * P, CH:],
                                osts[tbl][:, 1, :])
                        else:
                            nc.sync.dma_start(out_d[tb * P:(tb + 1) * P, :],
                                              osts[tbl][:])
                return run

            for tbl in range(NBC):
                for oc in range(2):
                    thunks.append(pair(tbl, oc))
            return thunks

        # ---- normalize Yt rows by the sums rows (ytp partition 0), store
        # into y2 via partition-shifting SBUF->SBUF DMAs.  Both heads of the
        # pair share one broadcast/reciprocal/multiply over [65, 2*CH]. ----
        def normalize_pair(g, qi, ytps, last=False):
            """The PSUM tiles are drained immediately (copies) so the pool
            slots free before the reciprocal chain completes.

            last=True (latency-critical final group): the PV window put the
            dims at partitions 0..63 and the sums at 64, so the whole chain
            runs with 32-aligned partition bases: sums row bounced to
            partition 0 (DVE copy), reciprocal, rank-1 f32r PE matmul as the
            partition broadcast, and a per-head multiply straight into the
            [128, CH] proj lhsT layout — no gpsimd broadcast and no shift
            DMAs.  Returns the ybf2 tile the final proj reads as lhsT."""
            if last:
                snc = work.tile([1, 2, CH], f32, tag="snc", bufs=1)
                for hh in range(2):
                    nc.vector.tensor_copy(out=snc[:, hh, :],
                                          in_=ytps[hh][HD:HD + 1, :])
                rcp32 = work.tile([1, 2, CH], f32, tag="rcp32", bufs=1)
                nc.vector.reciprocal_approx_fast(out=rcp32[:], in_=snc[:])
                rcp = work.tile([1, 2, CH], mdt, tag="rcp", bufs=1)
                nc.vector.tensor_copy(out=rcp[:], in_=rcp32[:])
                bnc2 = work.tile([P, CH], f32, tag="bnc2", bufs=1)
                for hh in range(2):
                    nc.scalar.copy(out=bnc2[hh * HD:(hh + 1) * HD, :],
                                   in_=ytps[hh][:HD, :])
                junk(10)
                rbst = psm.tile([P, 2 * CH], f32, tag="st", bufs=3,
                                name="rbst")
                for hh in range(2):
                    nc.tensor.matmul(
                        rbst[hh * HD:(hh + 1) * HD, :CH],
                        lhsT=ones_sb[:],
                        rhs=rcp[:, hh, :],
                        start=True, stop=True)
                ybf2 = work.tile([P, CH], mdt, tag="ybf2", bufs=1)
                for hh in range(2):
                    nc.vector.tensor_tensor(
                        out=ybf2[hh * HD:(hh + 1) * HD, :],
                        in0=bnc2[hh * HD:(hh + 1) * HD, :],
                        in1=rbst[hh * HD:(hh + 1) * HD, :CH], op=Alu.mult)
                return ybf2
            bnc = work.tile([HD + 1, 2, CH], f32, tag="bounce", bufs=2)
            nc.vector.tensor_copy(out=bnc[:, 0, :], in_=ytps[0][:])
            nc.vector.tensor_copy(out=bnc[:, 1, :], in_=ytps[1][:])
            # partition_broadcast / custom-DVE ops ignore the AP base
            # partition on HW; the sums rows are at partition 0 by layout.
            rb = work.tile([HD + 1, 2, CH], f32, tag="rb", bufs=2)
            nc.gpsimd.partition_broadcast(rb[:], bnc[0:1, :, :])
            nc.vector.reciprocal_approx_fast(out=rb[:], in_=rb[:])
            ybs = work.tile([HD + 1, 2, CH], mdt, tag="ybs", bufs=2)
            # row 0 computes sums * 1/sums; only rows 1..64 are stored.
            # (DVE partition base must be aligned, so operate on [0:65].)
            nc.vector.tensor_tensor(out=ybs[:], in0=bnc[:], in1=rb[:],
                                    op=Alu.mult)
            for hh in range(2):
                # shift each head's 64 rows to partitions hh*64 .. hh*64+63
                nc.sync.dma_start(y2[qi][hh * HD:(hh + 1) * HD, g, :],
                                  ybs[1:, hh, :])
            return None

        # ---- attention for one query chunk, with PE work injection ----
        def attention(qi, queue, start_at=0):
            nkb = NBC * (qi + 1)
            it = 0
            ybf2 = None
            for g in range(2):
                fin = qi == NCH - 1 and g == 1
                # final group: window [1, 66) puts dims at partitions 0..63
                # and the sums row at 64 (see normalize_pair fast path)
                voff = 1 if fin else 0
                ytps = [psm.tile([HD + 1, CH], f32, tag="yt", bufs=2,
                                 name=f"ytp{hh}") for hh in range(2)]

                def emit_pv(pts, kb, qoff):
                    for hh in range(2):
                        hl = 2 * g + hh
                        nc.tensor.matmul(
                            ytps[hh][:, qoff:],
                            lhsT=va[kb // NBC][:, kb % NBC,
                                               hl * VE + voff:
                                               hl * VE + voff + HD + 1],
                            rhs=pts[:, hh * CH + qoff:(hh + 1) * CH],
                            start=(kb == 0), stop=(kb == nkb - 1))

                pend = deque()
                for kb in range(nkb):
                    o4 = kb - NBC * qi          # 0..3 on the diagonal band
                    qoff = max(0, o4) * KB
                    stp = psm.tile([P, 2 * CH], f32, tag="st", bufs=3,
                                   name="stp")
                    for hh in range(2):
                        nc.tensor.matmul(
                            stp[:, hh * CH + qoff:(hh + 1) * CH],
                            lhsT=kt[kb // NBC][hh * HD:(hh + 1) * HD, g,
                                               (kb % NBC) * KB:(kb % NBC + 1) * KB],
                            rhs=qt[qi][hh * HD:(hh + 1) * HD, g, qoff:],
                            start=True, stop=True)
                    # PV runs TWO blocks behind ST: half the PVs otherwise
                    # stall on the exp->select->sem chain (~6us total); the
                    # extra iteration of slack lets every PV start hot.
                    if len(pend) >= 2:
                        emit_pv(*pend.popleft())
                    if queue and it >= start_at:
                        # on the final chunk, drain the queue ~8 iterations
                        # early so the DVE is clear of proj drains when the
                        # latency-critical fast-normalize chain starts
                        dby = 2 * nkb - (8 if qi == NCH - 1 else 0)
                        n = -(-len(queue) // max(1, dby - it))
                        for _ in range(min(n, len(queue))):
                            queue.popleft()()
                    it += 1
                    pts = work.tile([P, 2 * CH], mdt, tag="p", bufs=4,
                                    name="pts")
                    if o4 >= 0:
                        nc.scalar.activation(
                            out=pts.rearrange("p (h c) -> p h c", h=2)[:, :, qoff:],
                            in_=stp.rearrange("p (h c) -> p h c", h=2)[:, :, qoff:],
                            func=Exp, scale=SC)
                        # causal mask: zero the upper triangle of the
                        # 128-wide diagonal band of P (keep where q >= k,
                        # i.e. band column f >= partition p).  Runs on
                        # GpSimd so the DVE queue never delays exp or PV.
                        for hh in range(2):
                            nc.gpsimd.affine_select(
                                out=pts[:, hh * CH + qoff:hh * CH + qoff + KB],
                                in_=pts[:, hh * CH + qoff:hh * CH + qoff + KB],
                                compare_op=Alu.is_ge, fill=0.0,
                                base=0, channel_multiplier=-1,
                                pattern=[[1, KB]])
                    else:
                        nc.scalar.activation(out=pts[:], in_=stp[:],
                                             func=Exp, scale=SC)
                    pend.append((pts, kb, qoff))
                while pend:
                    emit_pv(*pend.popleft())
                r = normalize_pair(g, qi, ytps, last=fin)
                if r is not None:
                    ybf2 = r
            return ybf2

        # ---- emission schedule ----
        def junk(n):
            """Dependency-free matmuls bridging a known PE stall so the
            p-state stays at max while DMAs land."""
            jt = psm.tile([P, 2 * CH], f32, tag="st", bufs=3,
                          name="junk")[:, :CH]
            for i in range(n):
                nc.tensor.matmul(jt[:], lhsT=warm[:, :P], rhs=warm[:],
                                 start=(i == 0), stop=(i == n - 1))

        # phase A: only q-g0/k-g0 of chunk 0 up front; the v-groups and the
        # g1 projections are injected into attention(0) just ahead of need.
        # The initial weight/Xt DMAs land slower than the PE consumes, so
        # junk bridges keep the clock hot between the first groups.
        t0 = qkv_group_thunks(0)
        t0[0]()
        junk(12)
        t0[1]()
        junk(12)
        carry = []
        for qi in range(NCH):
            # carried-over proj pairs (long-ready) fill the first iterations
            # right at the chunk boundary, where the score-tile rotation
            # otherwise stalls on the exp backlog
            queue = deque(carry)
            carry = []
            if qi == 0:
                queue.extend([t0[4], t0[5], t0[2], t0[3], t0[6], t0[7]])
            if qi + 1 < NCH:
                queue.extend(qkv_group_thunks(qi + 1))
            if qi > 0:
                pp = proj_pair_thunks(qi - 1)
                if qi < NCH - 1:
                    queue.extend(pp[:4])
                    carry = pp[4:]
                else:
                    queue.extend(pp)
            ybf2 = attention(qi, queue, start_at=0)
            while queue:
                queue.popleft()()
        # final proj: yc=0 reads old y2 (ready), yc=1 reads ybf2 straight
        # from the fast normalize — the first pairs briefly wait on the
        # mult, the rest stream.  (The p-state bridge is emitted inside
        # normalize_pair's fast path.)
        junk(3)
        for th in proj_pair_thunks(NCH - 1, ybf2=ybf2):
            th()
    nc.compile()
    return nc


def _to_mm_dtype(a):
    if MM_DTYPE == "bf16":
        import ml_dtypes
        return np.ascontiguousarray(a).astype(ml_dtypes.bfloat16)
    return np.ascontiguousarray(a).astype(np.float32)


def make_in_maps(X, W_qkv, W_proj, t_len=T):
    """Host-side sharding: slice + pre-arrange weights per head group,
    transpose X.  Layouts match the SBUF tensors so every weight DMA is
    fully contiguous:
      wq/wk/wv [128, 8, 256]: [p, kc, m] = W[kc*128+p, cols][m]
      wp       [128, 2, C]:   [64*hh+d, yc, m] = W_proj[(2*yc+hh)*64+d, m]
    """
    in_maps = []
    NKC = C // P
    xts = [_to_mm_dtype(np.asarray(X[b, :t_len, :]).T
                        .reshape(NKC, P, t_len).transpose(1, 0, 2))
           for b in range(B)]
    for c in range(NCORES):
        b = c // (NCORES // B)
        h0 = HPC * (c % (NCORES // B))
        cols = slice(h0 * HD, (h0 + HPC) * HD)

        def warr(w):
            return _to_mm_dtype(
                np.ascontiguousarray(w).reshape(NKC, P, HPC * HD)
                .transpose(1, 0, 2))

        wp_c = np.ascontiguousarray(W_proj[cols, :])          # [256, C]
        wp2 = wp_c.reshape(2, 2, HD, C).transpose(1, 2, 0, 3).reshape(P, 2, C)
        in_maps.append({
            "xt": xts[b],
            "wq": warr(W_qkv[:, cols]),
            "wk": warr(W_qkv[:, C:][:, cols]),
            "wv": warr(W_qkv[:, 2 * C:][:, cols]),
            "wp": _to_mm_dtype(wp2),
        })
    return in_maps


_CACHE = {}
TRACE = False           # set True (e.g. from test.py) to capture an NTFF profile


def kernel(X, W_qkv, W_proj):
    import sys
    if "/opt/trn_rl_repo" not in sys.path:
        sys.path.insert(0, "/opt/trn_rl_repo")
    from concourse.bass_utils import run_bass_kernel_spmd

    X = np.asarray(X, dtype=np.float32)
    W_qkv = np.asarray(W_qkv, dtype=np.float32)
    W_proj = np.asarray(W_proj, dtype=np.float32)

    if "nc" not in _CACHE:
        _CACHE["nc"] = build_nc()
    nc = _CACHE["nc"]

    in_maps = make_in_maps(X, W_qkv, W_proj)
    res = run_bass_kernel_spmd(nc, in_maps, core_ids=list(range(NCORES)),
                               trace=TRACE)
    _CACHE["last"] = res
    out = np.empty((B, T, C), dtype=np.float32)
    ncb = NCORES // B
    for b in range(B):
        acc = res.results[b * ncb]["out"].astype(np.float32)
        for c in range(b * ncb + 1, (b + 1) * ncb):
            acc = acc + res.results[c]["out"].astype(np.float32)
        out[b] = acc
    return out

